# revision 13
# baseline (speedup 1.0000x reference)
"""Memristor linear layer kernel for 8 TRN2 NeuronCores.

The reference memristor crossbar computation collapses algebraically to
    out = x @ weights.T + bias
(the G_OFF offsets cancel in the pos/neg column subtraction and the k_G /
k_I scale factors cancel exactly), so the kernel computes the plain linear
layer.

Precision: fp32 operands are split on host into bf16 hi + bf16 lo halves;
the device computes hi*hi + hi*lo + lo*hi with fp32 PSUM accumulation
(~4e-6 relative error vs 3e-7 for native fp32) at full bf16 PE rate.

Sharding: tensor-parallel over the 1024 output features -> 128 per core.
Each core receives x.T (replicated) and its W.T column shard, pre-packed
on host into the exact SBUF layout [128 partitions, k_tile, free] so
every DMA moves per-partition-contiguous rows at line rate. Each core
computes its out.T shard [128, 256] = W_shard @ x.T + bias accumulated
over 8 K-chunks of 128 in PSUM. Host concatenates and transposes back.
"""

import os

import numpy as np

BATCH = 256
SIZE_IN = 1024
SIZE_OUT = 1024
N_CORES = 8
O_SHARD = SIZE_OUT // N_CORES  # 128
K_TILES = SIZE_IN // 128  # 8

_STATE = {}


def _build():
    import concourse.bass as bass
    import concourse.tile as tile
    from concourse import bacc, mybir

    f32 = mybir.dt.float32
    bf16 = mybir.dt.bfloat16
    n_warm = int(os.environ.get("WARMUP_MM", "6"))

    nc = bacc.Bacc(None, target_bir_lowering=False)

    # All tensors pre-packed on host to [128, ..., free] (partition major)
    # so every DMA descriptor is a large per-partition-contiguous run.
    xh_d = nc.declare_dram_parameter("xh", [128, K_TILES, BATCH], bf16, isOutput=False)
    xl_d = nc.declare_dram_parameter("xl", [128, K_TILES, BATCH], bf16, isOutput=False)
    whl_d = nc.declare_dram_parameter(
        "whl", [128, 2, K_TILES, O_SHARD], bf16, isOutput=False
    )
    b_d = nc.declare_dram_parameter("bias", [O_SHARD, 1], f32, isOutput=False)
    out_d = nc.declare_dram_parameter("out", [O_SHARD, BATCH], f32, isOutput=True)

    with tile.TileContext(nc) as tc:
        with (
            tc.tile_pool(name="sbuf", bufs=1) as pool,
            tc.tile_pool(name="psum", bufs=1, space="PSUM") as psum_pool,
        ):
            xh_s = pool.tile([128, K_TILES, BATCH], bf16)
            xl_s = pool.tile([128, K_TILES, BATCH], bf16)
            whl_s = pool.tile([128, 2, K_TILES, O_SHARD], bf16)
            b_s = pool.tile([O_SHARD, 1], f32)
            o_s = pool.tile([O_SHARD, BATCH], f32)
            pt = psum_pool.tile([O_SHARD, BATCH], f32)

            # PE warm-up: garbage matmuls into a scratch PSUM bank so the
            # HAM clock-gate releases (1.2 -> 2.4 GHz) while DMAs stream.
            if n_warm:
                warm_in = pool.tile([128, 512], bf16)
                warm_ps = psum_pool.tile([128, 512], f32)
                nc.vector.memset(warm_in[:], 0.0)
                for _ in range(n_warm):
                    nc.tensor.matmul(
                        warm_ps[:], warm_in[:, 0:128], warm_in[:], start=True,
                        stop=True,
                    )

            # The two HWDGE rings drain serially (~285 GB/s single stream),
            # so order everything on one ring by need: weights, x hi, x lo.
            # Each transfer is 512 KB -> 4 KB/partition descriptors.
            nc.sync.dma_start(out=whl_s[:], in_=whl_d[:])
            nc.sync.dma_start(out=xh_s[:], in_=xh_d[:])
            nc.sync.dma_start(out=xl_s[:], in_=xl_d[:])
            # bias via the gpsimd SWDGE path (off both HWDGE rings)
            nc.gpsimd.dma_start(out=b_s[:], in_=b_d[:])

            # 24 accumulating matmuls: pass A interleaves hi*hi and lo*hi
            # per k (whl + xh gate), pass B is hi*lo (x lo halves gate).
            plan = []
            for k in range(K_TILES):
                plan.append((whl_s, 0, xh_s, k))
                plan.append((whl_s, 1, xh_s, k))
            for k in range(K_TILES):
                plan.append((whl_s, 0, xl_s, k))
            for i, (ws, hl, xs, k) in enumerate(plan):
                nc.tensor.matmul(
                    pt[:],
                    ws[:, hl, k, :],
                    xs[:, k, :],
                    start=(i == 0),
                    stop=(i == len(plan) - 1),
                )

            nc.vector.tensor_scalar_add(out=o_s[:], in0=pt[:], scalar1=b_s[:])
            # out halves on both HWDGE rings so the completion receipts
            # (~1 us each to HBM) overlap
            nc.sync.dma_start(out=out_d[:, 0:BATCH // 2], in_=o_s[:, 0:BATCH // 2])
            nc.scalar.dma_start(out=out_d[:, BATCH // 2 :], in_=o_s[:, BATCH // 2 :])

    nc.compile()
    return nc


def _install_ntff_hook_shim():
    """The agent image's antenv lacks axon_hooks; recreate it so
    run_bass_kernel_spmd(trace=True) can capture NTFF profiles."""
    import sys
    import types

    if "antenv.axon_hooks" in sys.modules:
        return
    mod = types.ModuleType("antenv.axon_hooks")
    mod._HOOK = None

    def set_axon_ntff_profile_hook(hook):
        mod._HOOK = hook

    def get_axon_ntff_profile_hook():
        return mod._HOOK

    mod.set_axon_ntff_profile_hook = set_axon_ntff_profile_hook
    mod.get_axon_ntff_profile_hook = get_axon_ntff_profile_hook
    sys.modules["antenv.axon_hooks"] = mod
    try:
        from trn_agent_boot.trn_boot import _ntff_profile_via_ctypes

        mod._HOOK = _ntff_profile_via_ctypes("/opt/axon/libaxon_pjrt.so")
    except Exception:
        pass


def _split_pack(a_t: np.ndarray, ncols: int):
    """[SIZE_IN, ncols] f32 -> two bf16 arrays packed as [128, K_TILES, ncols]."""
    import ml_dtypes

    hi = a_t.astype(ml_dtypes.bfloat16)
    lo = (a_t - hi.astype(np.float32)).astype(ml_dtypes.bfloat16)

    def pack(v):
        return np.ascontiguousarray(
            v.reshape(K_TILES, 128, ncols).transpose(1, 0, 2)
        )

    return pack(hi), pack(lo)


def _split_pack_w(w_t: np.ndarray):
    """[SIZE_IN, O_SHARD] f32 -> one bf16 array [128, 2, K_TILES, O_SHARD]
    holding the hi and lo halves contiguously per partition."""
    hi, lo = _split_pack(w_t, O_SHARD)
    return np.ascontiguousarray(np.stack([hi, lo], axis=1))


def kernel(x: np.ndarray, weights: np.ndarray, bias: np.ndarray) -> np.ndarray:
    from concourse.bass_utils import run_bass_kernel_spmd

    if "nc" not in _STATE:
        _STATE["nc"] = _build()
    nc = _STATE["nc"]

    x = np.asarray(x, dtype=np.float32)
    weights = np.asarray(weights, dtype=np.float32)
    bias = np.asarray(bias, dtype=np.float32)

    xt = np.ascontiguousarray(x.T)  # [SIZE_IN, BATCH] f32
    xh, xl = _split_pack(xt, BATCH)
    wt = np.ascontiguousarray(weights.T)  # [SIZE_IN, SIZE_OUT] f32

    in_maps = []
    for c in range(N_CORES):
        sl = slice(c * O_SHARD, (c + 1) * O_SHARD)
        in_maps.append(
            {
                "xh": xh,
                "xl": xl,
                "whl": _split_pack_w(np.ascontiguousarray(wt[:, sl])),
                "bias": np.ascontiguousarray(bias[sl]).reshape(O_SHARD, 1),
            }
        )

    trace = os.environ.get("BASS_PROBLEM_TRACE", "0") == "1"
    if trace:
        _install_ntff_hook_shim()
    res = run_bass_kernel_spmd(
        nc, in_maps, core_ids=list(range(N_CORES)), trace=trace
    )
    _STATE["last_results"] = res

    out_t = np.concatenate(
        [np.asarray(res.results[c]["out"]) for c in range(N_CORES)], axis=0
    )  # [SIZE_OUT, BATCH]
    return np.ascontiguousarray(out_t.T).astype(np.float32, copy=False)


# revision 14
# speedup vs baseline: 1.0813x; 1.0813x over previous
"""Memristor linear layer kernel for 8 TRN2 NeuronCores.

The reference memristor crossbar computation collapses algebraically to
    out = x @ weights.T + bias
(the G_OFF offsets cancel in the pos/neg column subtraction and the k_G /
k_I scale factors cancel exactly), so the kernel computes the plain linear
layer.

Precision: fp32 operands are split on host into bf16 hi + bf16 lo halves;
the device computes hi*hi + hi*lo + lo*hi with fp32 PSUM accumulation
(~4e-6 relative error vs 3e-7 for native fp32) at full bf16 PE rate.

Sharding: tensor-parallel over the 1024 output features -> 128 per core.
Each core receives x.T (replicated) and its W.T column shard, pre-packed
on host into the exact SBUF layout [128 partitions, k_tile, free] so
every DMA moves per-partition-contiguous rows at line rate. Each core
computes its out.T shard [128, 256] = W_shard @ x.T + bias accumulated
over 8 K-chunks of 128 in PSUM. Host concatenates and transposes back.
"""

import os

import numpy as np

BATCH = 256
SIZE_IN = 1024
SIZE_OUT = 1024
N_CORES = 8
O_SHARD = SIZE_OUT // N_CORES  # 128
K_TILES = SIZE_IN // 128  # 8

_STATE = {}


def _build():
    import concourse.bass as bass
    import concourse.tile as tile
    from concourse import bacc, mybir

    f32 = mybir.dt.float32
    bf16 = mybir.dt.bfloat16
    n_warm = int(os.environ.get("WARMUP_MM", "6"))

    nc = bacc.Bacc(None, target_bir_lowering=False)

    # All tensors pre-packed on host to [128, ..., free] (partition major)
    # so every DMA descriptor is a large per-partition-contiguous run.
    xh_d = nc.declare_dram_parameter("xh", [128, K_TILES, BATCH], bf16, isOutput=False)
    xl_d = nc.declare_dram_parameter("xl", [128, K_TILES, BATCH], bf16, isOutput=False)
    whl_d = nc.declare_dram_parameter(
        "whl", [128, 2, K_TILES, O_SHARD], bf16, isOutput=False
    )
    b_d = nc.declare_dram_parameter("bias", [O_SHARD, 1], f32, isOutput=False)
    out_d = nc.declare_dram_parameter("out", [O_SHARD, BATCH], f32, isOutput=True)

    with tile.TileContext(nc) as tc:
        with (
            tc.tile_pool(name="sbuf", bufs=1) as pool,
            tc.tile_pool(name="psum", bufs=1, space="PSUM") as psum_pool,
        ):
            xh_s = pool.tile([128, K_TILES, BATCH], bf16)
            xl_s = pool.tile([128, K_TILES, BATCH], bf16)
            whl_s = pool.tile([128, 2, K_TILES, O_SHARD], bf16)
            b_s = pool.tile([O_SHARD, 1], f32)
            o_s = pool.tile([O_SHARD, BATCH], f32)
            pt = psum_pool.tile([O_SHARD, BATCH], f32)

            # PE warm-up: garbage matmuls into a scratch PSUM bank so the
            # HAM clock-gate releases (1.2 -> 2.4 GHz) while DMAs stream.
            if n_warm:
                warm_in = pool.tile([128, 512], bf16)
                warm_ps = psum_pool.tile([128, 512], f32)
                nc.vector.memset(warm_in[:], 0.0)
                for _ in range(n_warm):
                    nc.tensor.matmul(
                        warm_ps[:], warm_in[:, 0:128], warm_in[:], start=True,
                        stop=True,
                    )

            # 512 KB transfers (4 KB/partition descriptors). The HWDGE
            # drains roughly serially across rings in issue order, but
            # same-ring back-to-back transfers lose ~20%, so alternate:
            # weights on sync, x hi + x lo on scalar.
            nc.sync.dma_start(out=whl_s[:], in_=whl_d[:])
            nc.scalar.dma_start(out=xh_s[:], in_=xh_d[:])
            nc.scalar.dma_start(out=xl_s[:], in_=xl_d[:])
            # bias via the gpsimd SWDGE path (off both HWDGE rings)
            nc.gpsimd.dma_start(out=b_s[:], in_=b_d[:])

            # 24 accumulating matmuls: pass A interleaves hi*hi and lo*hi
            # per k (whl + xh gate), pass B is hi*lo (x lo halves gate).
            plan = []
            for k in range(K_TILES):
                plan.append((whl_s, 0, xh_s, k))
                plan.append((whl_s, 1, xh_s, k))
            for k in range(K_TILES):
                plan.append((whl_s, 0, xl_s, k))
            for i, (ws, hl, xs, k) in enumerate(plan):
                nc.tensor.matmul(
                    pt[:],
                    ws[:, hl, k, :],
                    xs[:, k, :],
                    start=(i == 0),
                    stop=(i == len(plan) - 1),
                )

            nc.vector.tensor_scalar_add(out=o_s[:], in0=pt[:], scalar1=b_s[:])
            # out halves on both HWDGE rings so the completion receipts
            # (~1 us each to HBM) overlap
            nc.sync.dma_start(out=out_d[:, 0:BATCH // 2], in_=o_s[:, 0:BATCH // 2])
            nc.scalar.dma_start(out=out_d[:, BATCH // 2 :], in_=o_s[:, BATCH // 2 :])

    nc.compile()
    return nc


def _install_ntff_hook_shim():
    """The agent image's antenv lacks axon_hooks; recreate it so
    run_bass_kernel_spmd(trace=True) can capture NTFF profiles."""
    import sys
    import types

    if "antenv.axon_hooks" in sys.modules:
        return
    mod = types.ModuleType("antenv.axon_hooks")
    mod._HOOK = None

    def set_axon_ntff_profile_hook(hook):
        mod._HOOK = hook

    def get_axon_ntff_profile_hook():
        return mod._HOOK

    mod.set_axon_ntff_profile_hook = set_axon_ntff_profile_hook
    mod.get_axon_ntff_profile_hook = get_axon_ntff_profile_hook
    sys.modules["antenv.axon_hooks"] = mod
    try:
        from trn_agent_boot.trn_boot import _ntff_profile_via_ctypes

        mod._HOOK = _ntff_profile_via_ctypes("/opt/axon/libaxon_pjrt.so")
    except Exception:
        pass


def _split_pack(a_t: np.ndarray, ncols: int):
    """[SIZE_IN, ncols] f32 -> two bf16 arrays packed as [128, K_TILES, ncols]."""
    import ml_dtypes

    hi = a_t.astype(ml_dtypes.bfloat16)
    lo = (a_t - hi.astype(np.float32)).astype(ml_dtypes.bfloat16)

    def pack(v):
        return np.ascontiguousarray(
            v.reshape(K_TILES, 128, ncols).transpose(1, 0, 2)
        )

    return pack(hi), pack(lo)


def _split_pack_w(w_t: np.ndarray):
    """[SIZE_IN, O_SHARD] f32 -> one bf16 array [128, 2, K_TILES, O_SHARD]
    holding the hi and lo halves contiguously per partition."""
    hi, lo = _split_pack(w_t, O_SHARD)
    return np.ascontiguousarray(np.stack([hi, lo], axis=1))


def kernel(x: np.ndarray, weights: np.ndarray, bias: np.ndarray) -> np.ndarray:
    from concourse.bass_utils import run_bass_kernel_spmd

    if "nc" not in _STATE:
        _STATE["nc"] = _build()
    nc = _STATE["nc"]

    x = np.asarray(x, dtype=np.float32)
    weights = np.asarray(weights, dtype=np.float32)
    bias = np.asarray(bias, dtype=np.float32)

    xt = np.ascontiguousarray(x.T)  # [SIZE_IN, BATCH] f32
    xh, xl = _split_pack(xt, BATCH)
    wt = np.ascontiguousarray(weights.T)  # [SIZE_IN, SIZE_OUT] f32

    in_maps = []
    for c in range(N_CORES):
        sl = slice(c * O_SHARD, (c + 1) * O_SHARD)
        in_maps.append(
            {
                "xh": xh,
                "xl": xl,
                "whl": _split_pack_w(np.ascontiguousarray(wt[:, sl])),
                "bias": np.ascontiguousarray(bias[sl]).reshape(O_SHARD, 1),
            }
        )

    trace = os.environ.get("BASS_PROBLEM_TRACE", "0") == "1"
    if trace:
        _install_ntff_hook_shim()
    res = run_bass_kernel_spmd(
        nc, in_maps, core_ids=list(range(N_CORES)), trace=trace
    )
    _STATE["last_results"] = res

    out_t = np.concatenate(
        [np.asarray(res.results[c]["out"]) for c in range(N_CORES)], axis=0
    )  # [SIZE_OUT, BATCH]
    return np.ascontiguousarray(out_t.T).astype(np.float32, copy=False)
